# revision 2
# baseline (speedup 1.0000x reference)
"""AeroForceLoss Trainium2 kernel.

Computes, over prediction/target [N,4], normals [N,3], areas [N,1] with
N = B*S (B=16 segments of S=131072 points):

    diff = prediction - target
    base_loss = mean(diff^2)
    vec = (diff[...,0:1] + diff[...,1:4]) * normals * areas   (per segment)
    d_fa[b] = sum_s vec                                       [B,3]
    force_loss = mean_b ||d_fa[b]||_2
    out = base_loss + 0.1 * force_loss

Data-parallel across 8 NeuronCores: core i handles segments 2i, 2i+1.
Each core reduces its 12.6 MB shard to a [128, 16] accumulator tile
(per-chunk sum-of-squares columns + per-(chunk,K) force partial sums);
the host gathers the 8 tiny tiles and does the final scalar math.
"""

import numpy as np

import concourse.bacc as bacc
import concourse.mybir as mybir
import concourse.tile as tile
from concourse import bass_utils

FORCE_LOSS_WEIGHT = 0.1

B = 16  # segments (batch)
S = 131072  # points per segment
N_CORES = 8
SEGS = B // N_CORES  # segments per core = 2
P = 128  # SBUF partitions
CH = 512  # points per partition per chunk

F32 = mybir.dt.float32


def build_nc(segs=SEGS, s_len=S, ch=CH):
    """Build the per-core SPMD Bass module.

    Inputs (per core): pred/targ [segs*s_len, 4], nrm [segs*s_len, 3],
    area [segs*s_len, 1], all f32. Output: acc_out [128, 4*ntot] where
    ntot = total chunks; col g in [0, ntot) holds per-partition
    sum(diff^2) of chunk g, col ntot+3g+K holds per-partition
    sum((d0+dK)*nK*a) of chunk g for K in 0..2.
    """
    q = s_len // P  # points per partition per segment
    assert s_len % P == 0 and q % ch == 0
    nch = q // ch  # chunks per segment
    ntot = segs * nch

    nc = bacc.Bacc("TRN2", target_bir_lowering=False, debug=False)
    npts = segs * s_len
    pred = nc.dram_tensor("pred", [npts, 4], F32, kind="ExternalInput").ap()
    targ = nc.dram_tensor("targ", [npts, 4], F32, kind="ExternalInput").ap()
    nrm = nc.dram_tensor("nrm", [npts, 3], F32, kind="ExternalInput").ap()
    area = nc.dram_tensor("area", [npts, 1], F32, kind="ExternalInput").ap()
    acc_dram = nc.dram_tensor("acc_out", [P, 4 * ntot], F32, kind="ExternalOutput").ap()

    # Partition p of segment s holds points [s*s_len + p*q, ... + q).
    pred_v = pred.rearrange("(s p j) c -> s p (j c)", s=segs, p=P)
    targ_v = targ.rearrange("(s p j) c -> s p (j c)", s=segs, p=P)
    nrm_v = nrm.rearrange("(s p j) c -> s p (j c)", s=segs, p=P)
    area_v = area.rearrange("(s p j) c -> s p (j c)", s=segs, p=P)

    with tile.TileContext(nc) as tc:
        with (
            tc.tile_pool(name="loads", bufs=3) as loads,
            tc.tile_pool(name="work", bufs=2) as work,
            tc.tile_pool(name="accp", bufs=1) as accp,
        ):
            acc = accp.tile([P, 4 * ntot], F32)
            for s in range(segs):
                for k in range(nch):
                    g = s * nch + k
                    tp = loads.tile([P, ch * 4], F32, tag="tp")
                    tt = loads.tile([P, ch * 4], F32, tag="tt")
                    tn = loads.tile([P, ch * 3], F32, tag="tn")
                    ta = loads.tile([P, ch], F32, tag="ta")
                    nc.sync.dma_start(out=tp, in_=pred_v[s, :, k * ch * 4:(k + 1) * ch * 4])
                    nc.sync.dma_start(out=tt, in_=targ_v[s, :, k * ch * 4:(k + 1) * ch * 4])
                    nc.sync.dma_start(out=tn, in_=nrm_v[s, :, k * ch * 3:(k + 1) * ch * 3])
                    nc.sync.dma_start(out=ta, in_=area_v[s, :, k * ch:(k + 1) * ch])

                    # diff = pred - targ  (DVE)
                    td = work.tile([P, ch * 4], F32, tag="td")
                    nc.vector.tensor_sub(td, tp, tt)

                    # base loss: sum(diff^2) into acc[:, g]  (ACT, fused)
                    tsq = work.tile([P, ch * 4], F32, tag="tsq")
                    nc.scalar.activation(
                        out=tsq,
                        in_=td,
                        func=mybir.ActivationFunctionType.Square,
                        accum_out=acc[:, g:g + 1],
                    )

                    # m = normals * areas (broadcast area over 3 comps)  (DVE)
                    tn3 = tn.rearrange("p (j c) -> p j c", c=3)
                    ta_b = ta.unsqueeze(2).broadcast_to((P, ch, 3))
                    tm = work.tile([P, ch * 3], F32, tag="tm")
                    tm3 = tm.rearrange("p (j c) -> p j c", c=3)
                    nc.vector.tensor_mul(tm3, tn3, ta_b)

                    # sK = d0 + dK  (DVE, d0 broadcast over K)
                    td4 = td.rearrange("p (j c) -> p j c", c=4)
                    d0b = td4[:, :, 0:1].broadcast_to((P, ch, 3))
                    ts = work.tile([P, ch * 3], F32, tag="ts")
                    ts3 = ts.rearrange("p (j c) -> p j c", c=3)
                    nc.vector.tensor_add(ts3, d0b, td4[:, :, 1:4])

                    # per K: w = sK*mK, acc[:, ntot+3g+K] = sum(w)  (DVE, fused)
                    tw = work.tile([P, ch * 3], F32, tag="tw")
                    tw3 = tw.rearrange("p (j c) -> p j c", c=3)
                    for K in range(3):
                        col = ntot + 3 * g + K
                        nc.vector.scalar_tensor_tensor(
                            out=tw3[:, :, K:K + 1],
                            in0=ts3[:, :, K:K + 1],
                            scalar=0.0,
                            in1=tm3[:, :, K:K + 1],
                            op0=mybir.AluOpType.add,
                            op1=mybir.AluOpType.mult,
                            accum_out=acc[:, col:col + 1],
                        )
            nc.sync.dma_start(out=acc_dram, in_=acc)
    nc.compile()
    return nc


_NC_CACHE = {}


def _get_nc():
    if "nc" not in _NC_CACHE:
        _NC_CACHE["nc"] = build_nc()
    return _NC_CACHE["nc"]


def combine_host(accs, segs=SEGS, s_len=S, ch=CH):
    """accs: [n_cores, 128, 4*ntot] -> scalar loss (float64 math)."""
    n_cores = accs.shape[0]
    q = s_len // P
    nch = q // ch
    ntot = segs * nch
    ss = accs[:, :, :ntot].sum(dtype=np.float64)
    base = ss / (n_cores * segs * s_len * 4)
    f = accs[:, :, ntot:].reshape(n_cores, P, segs, nch, 3)
    f = f.sum(axis=(1, 3), dtype=np.float64)  # [n_cores, segs, 3]
    norms = np.sqrt((f * f).sum(axis=-1))  # [n_cores, segs]
    force = norms.mean()
    return base + FORCE_LOSS_WEIGHT * force


def kernel(prediction, target, normals, areas, batch_size=B, sim_len=S, **_):
    assert int(batch_size) == B and int(sim_len) == S
    prediction = np.ascontiguousarray(np.asarray(prediction, dtype=np.float32))
    target = np.ascontiguousarray(np.asarray(target, dtype=np.float32))
    normals = np.ascontiguousarray(np.asarray(normals, dtype=np.float32))
    areas = np.ascontiguousarray(np.asarray(areas, dtype=np.float32))

    nc = _get_nc()
    rows = SEGS * S
    in_maps = [
        {
            "pred": prediction[i * rows:(i + 1) * rows],
            "targ": target[i * rows:(i + 1) * rows],
            "nrm": normals[i * rows:(i + 1) * rows],
            "area": areas[i * rows:(i + 1) * rows],
        }
        for i in range(N_CORES)
    ]
    res = bass_utils.run_bass_kernel_spmd(nc, in_maps, core_ids=list(range(N_CORES)))
    accs = np.stack([r["acc_out"] for r in res.results])
    return np.float32(combine_host(accs))
